# revision 48
# baseline (speedup 1.0000x reference)
"""Masked video loss kernel for TRN2 (8 NeuronCores, SPMD) — fp8 DoubleRow.

Exploits: (1) the decoder input is spatially constant, so the three SAME
3x3x3 convs take at most 7x7 distinct values per (b,c,t) — evaluated
exactly on a 7x7 class grid; (2) fp8 DoubleRow matmuls (two 128-row
contraction planes per pass at 0.5 cycles/output-col); (3) a t-split
across the two cores of each batch: core th computes recon only for its
8 frames (plus conv halo), with the shift encoded purely in host-side
data placement so the SPMD program is identical on all cores.

Per-core program:
  encoder  feats[t,d] = (masked obs)·W_enc^T + b_enc for 12 local frame
           slots (11 real + 1 zero pad), fp8 DR over 96 k-chunks, with
           d-halves pipelined against the W_enc DMA stream.
  bcast    xpad1[d,(t,9,9)] = feats broadcast to the padded class grid,
           done on the PE with a constant 0/1-ish matrix E (entries 1/S,
           which also undoes the fp8 weight scaling S).
  conv1-3  each SAME conv = 27 taps paired into fp8-DR plane pairs over
           overlapping windows of the padded (13-slot x 9 x 9) volume.
           Weight scale S folded out via activation scale=1/S.
  stats    masked-MSE per-class statistics (cnt, s1, s2) from bf16 obs,
           own t-half only, on DVE + 2 small PE matmuls.
Host assembles the scalar loss from per-class tensors (~10k flops).

mask zeroing: sum_masked (r-o)^2 = r^2*cnt - 2 r*s1 + s2 per class.
"""

import sys

sys.path.insert(0, "/opt/trn_rl_repo")

from contextlib import ExitStack  # noqa: E402

import numpy as np  # noqa: E402

import concourse.bacc as bacc  # noqa: E402
import concourse.mybir as mybir  # noqa: E402
import concourse.tile as tile  # noqa: E402
from concourse import bass_utils  # noqa: E402
from concourse.ap import AP  # noqa: E402

B, T, C, H, W = 4, 16, 3, 64, 64
D = 256
X = C * H * W  # 12288
NCORES = 8
S = 64.0  # fp8 weight scale (keeps 0.02-scale weights out of e4m3 subnormals)

F32 = mybir.dt.float32
BF16 = mybir.dt.bfloat16
FP8 = mybir.dt.float8e4
DR = mybir.MatmulPerfMode.DoubleRow
AF = mybir.ActivationFunctionType

NF = 12  # local feats slots (11 real frames + 1 zero pad)
NT = 10  # conv out t-slots per stage
S23 = 13  # xpad2/xpad3 t-slots (extra zero slot for the Z-plane trick)
FLAT = 81  # 9x9 padded spatial grid, flattened
NV = NT * 49  # 490 valid out cols per stage

# conv out t-splits (t0, nt): each split's rhs free = 2*nt*61 <= 512.
# conv2's tiny first split lets it start after conv1's first act alone.
SPLITS = [(0, 4), (4, 4), (8, 2)]
SPLITS2 = [(0, 4), (4, 4), (8, 2)]

# DR plane-pair tables. Pair = (tapA, tapB, plane_stride); tapB None => zero
# weights in plane B (rhs reads base+stride window, in-bounds by design).
KHW = [(kh, kw) for kh in range(3) for kw in range(3)]


def tap_off(t):
    kt, kh, kw = t
    return kt * FLAT + kh * 9 + kw


# conv2/conv3: 27 taps -> 14 pairs (one Z plane)
PAIRS23 = (
    [((0, kh, kw), (1, kh, kw), FLAT) for kh, kw in KHW]
    + [((2, 0, kw), (2, 1, kw), 9) for kw in range(3)]
    + [((2, 2, 0), (2, 2, 1), 1), ((2, 2, 2), None, FLAT)]
)
# conv1 per-kd local pairs: 13 (covers all taps except (2,2,2))
PAIRS1L = (
    [((0, kh, kw), (1, kh, kw), FLAT) for kh, kw in KHW]
    + [((2, 0, kw), (2, 1, kw), 9) for kw in range(3)]
    + [((2, 2, 0), (2, 2, 1), 1)]
)
NP1 = 2 * len(PAIRS1L) + 1  # 27: kd0 locals + kd1 locals + one cross-kd pair

CLS_BOUNDS = [0, 1, 2, 3, H - 3, H - 2, H - 1, H]
NCLS = 7
TTL = 4  # local tt pairs (own t-half = 8 frames = 4 frame-pairs)

W1COLS = NP1 * 256  # 6912
W2BASE = W1COLS
W2COLS = len(PAIRS23) * 128  # 1792
W3BASE = W2BASE + W2COLS
W3COLS = len(PAIRS23) * 32  # padded for DR stride%16
WCCOLS = W3BASE + W3COLS


def _win(tile_ap, off, pstride, nt, npart=None):
    """rhs window AP [p][plane 2][t nt][flat 61]: 3 free dims (HW ISA limit).

    The 61-span covers the 7x7 window in the 9-col row layout, including 12
    junk columns (j in {7,8} positions) that downstream activations skip."""
    pdim = [tile_ap.ap[0][0], npart or tile_ap.ap[0][1]]
    return AP(
        tile_ap.tensor,
        tile_ap.offset + off,
        [pdim, [pstride, 2], [9, 9 * nt - 2], [1, 7]],
    )


def _xwin(tile_ap, t0, nt, npart=None):
    """xpad write AP [p][t nt][i 7][j 7] at interior of slots t0+1.."""
    pdim = [tile_ap.ap[0][0], npart or tile_ap.ap[0][1]]
    return AP(
        tile_ap.tensor,
        tile_ap.offset + (t0 + 1) * FLAT + 10,
        [pdim, [FLAT, nt], [9, 7], [1, 7]],
    )


def _emit(nc, a_in, a_out):
    ctx = ExitStack()
    tc = tile.TileContext(nc)
    with tc, ctx:
        io = ctx.enter_context(tc.tile_pool(name="io", bufs=1))
        wpool = ctx.enter_context(tc.tile_pool(name="wpool", bufs=1))
        ps_enc = ctx.enter_context(tc.tile_pool(name="ps_enc", bufs=2, space="PSUM"))
        ps_bc = ctx.enter_context(tc.tile_pool(name="ps_bc", bufs=2, space="PSUM"))
        ps_cv = ctx.enter_context(tc.tile_pool(name="ps_cv", bufs=3, space="PSUM"))
        ps_wm = ctx.enter_context(tc.tile_pool(name="ps_wm", bufs=1, space="PSUM"))

        # ---------------- input DMAs ----------------
        # warm-up operand: Pool memset, no DMA dependency
        wz = io.tile([1, 256], BF16)
        nc.gpsimd.memset(wz[:], 0.0)
        xt = io.tile([128, 1536], FP8)
        nc.sync.dma_start(xt[:], a_in["xt"])
        # t padded 12->16: DoubleRow ldweights needs plane stride % 16 == 0
        xt_v = xt[:].rearrange("p (g a t) -> p g a t", g=48, a=2)[:, :, :, 0:12]
        scb = io.tile([1, 272], BF16)  # [0:256) S*b_enc | [256:268) flag
        nc.sync.dma_start(scb[:], a_in["sflag"])

        # persistent SBUF tiles
        wc = io.tile([128, WCCOLS], FP8)
        featsTb = io.tile([12, 256], BF16)
        xpad1 = io.tile([128, 2 * NF * FLAT], FP8)  # [kd 2][t 12][81]
        xpad2 = io.tile([128, S23 * FLAT], FP8)
        xpad3 = io.tile([64, S23 * FLAT], FP8)
        recon_sb = io.tile([3, NV], F32)
        outv = io.tile([128, 120], F32)
        big2_sb = io.tile([128, 1040], BF16)
        cons = io.tile([128, 20], F32)
        emat = io.tile([12, 972], BF16)
        b1_sb = cons[:, 0:1]
        b2_sb = cons[0:64, 1:2]
        b3_sb = cons[0:3, 2:3]
        rhT_sb = cons[:, 3:17]
        nc.gpsimd.memset(xpad2[:], 0.0)
        nc.gpsimd.memset(xpad3[:], 0.0)
        nc.gpsimd.memset(outv[:], 0.0)

        # warm/keep-alive PSUM bank also hosts the tiny stats matmuls
        warm = ps_wm.tile([14, 368], F32)
        pv1 = warm[0:14, 256:340]
        pvc = warm[0:14, 340:368]

        def ka(n, cols=256):
            for _ in range(n):
                nc.tensor.matmul(
                    warm[0:2, 0:cols], wz[0:1, 0:2], wz[0:1, 0:cols],
                    start=True, stop=True,
                )

        # ---------------- PE warm-up during first DMAs ----------------
        ka(12)

        # ---------------- encoder, d-half pipelined against W_enc DMA ----
        featsT_ps = [None, None]
        wtiles = {}

        def enc_chunk(dh, g0, ng, eng=None):
            wk = wpool.tile(
                [128, ng * 256], FP8, name=f"wk{dh}{g0}", tag=f"wk{dh}{g0}"
            )
            base = dh * 12288 + g0 * 256
            (eng or nc.sync).dma_start(
                wk[:], a_in["wenc"][:, base : base + ng * 256]
            )
            wtiles[(dh, g0)] = (wk, ng)

        def enc_mms(dh, g0, stop=False):
            wk, ng = wtiles[(dh, g0)]
            wv = wk[:].rearrange("p (g a d) -> p g a d", g=ng, a=2)
            for gl in range(ng):
                g = g0 + gl
                nc.tensor.matmul(
                    featsT_ps[dh][:],
                    xt_v[:, g, :, :],
                    wv[:, gl, :, :],
                    start=(g == 0),
                    stop=(stop and gl == ng - 1),
                    perf_mode=DR,
                )

        def conv_chain(ps_t, split, pairs, wviews, xtile, kd_offs, npart,
                       start, stop):
            t0, nt = split
            n = len(pairs)
            for pi, (ta, tb, pstride) in enumerate(pairs):
                rhs = _win(
                    xtile[:], kd_offs[pi] + tap_off(ta) + t0 * FLAT, pstride,
                    nt, npart,
                )
                nc.tensor.matmul(
                    ps_t[:], wviews[pi], rhs,
                    start=(start and pi == 0),
                    stop=(stop and pi == n - 1),
                    perf_mode=DR,
                )

        # --- d-half 0 encoder (DMA-paced) ---
        featsT_ps[0] = ps_enc.tile([12, 128], F32, name="fe0", tag="fe")
        enc_chunk(0, 0, 12, eng=nc.scalar)
        enc_chunk(0, 12, 12)
        enc_mms(0, 0)
        ka(3, 128)
        enc_chunk(0, 24, 12)
        enc_mms(0, 12)
        ka(3, 128)
        enc_chunk(0, 36, 12)
        enc_mms(0, 24)
        ka(3, 128)
        nc.sync.dma_start(emat[:], a_in["ebmat"])
        # b_enc via per-core real-frame flag row (zero on pad frames)
        nc.tensor.matmul(
            featsT_ps[0][:], scb[0:1, 256:268], scb[0:1, 0:128],
            start=False, stop=False,
        )
        enc_mms(0, 36, stop=True)
        nc.vector.tensor_copy(featsTb[:, 0:128], featsT_ps[0][:])

        # --- bcast kd0 ---
        for h in range(2):
            bc = ps_bc.tile([128, 486], F32, name=f"bc0{h}", tag="bc")
            nc.tensor.matmul(
                bc[:], featsTb[:, 0:128], emat[:, h * 486 : (h + 1) * 486],
                start=True, stop=True,
            )
            nc.scalar.activation(
                xpad1[:, h * 486 : (h + 1) * 486], bc[:], AF.Identity
            )
        ka(3)
        featsT_ps[1] = ps_enc.tile([12, 128], F32, name="fe1", tag="fe")

        # --- conv1 kd0 setup (mms run post-stream, in the bcast-hop idle) ---
        p1 = [ps_cv.tile([128, (9 * nt - 2) * 7], F32, name=f"p1{si}", tag="cv")
              for si, (t0, nt) in enumerate(SPLITS)]

        def w1view(pi):
            return wc[:, pi * 256 : (pi + 1) * 256].rearrange(
                "p (a m) -> p a m", a=2
            )

        w1v_kd0 = [w1view(pi) for pi in range(13)]
        w1v_kd1 = [w1view(26)] + [w1view(13 + pi) for pi in range(13)]

        # --- stats payload + stats compute (runs mid-stream on DVE) ---
        obs_st = big2_sb[:, 0:768]
        mask_st = big2_sb[:, 768:1024]
        vO = obs_st.rearrange("p (tt c w) -> p tt c w", tt=TTL, c=C)
        vM = mask_st.rearrange("p (tt w) -> p tt w", tt=TTL)
        mo = io.tile([128, TTL * C * W], BF16)
        vmo = mo[:].rearrange("p (tt c w) -> p tt c w", tt=TTL, c=C)
        for c in range(C):
            nc.vector.tensor_mul(vmo[:, :, c, :], vO[:, :, c, :], vM[:])
        mo2 = io.tile([128, TTL * C * W], BF16)
        nc.vector.tensor_mul(mo2[:], mo[:], obs_st)
        nc.vector.reduce_sum(outv[:, 112:113], mo2[:], axis=mybir.AxisListType.X)
        U1 = io.tile([128, TTL * C * NCLS], F32)
        vU1 = U1[:].rearrange("p (tt c j) -> p tt c j", tt=TTL, c=C)
        Uc = io.tile([128, TTL * NCLS], F32)
        vUc = Uc[:].rearrange("p (tt j) -> p tt j", tt=TTL)
        nc.vector.tensor_copy(vU1[:, :, :, 0:3], vmo[:, :, :, 0:3])
        nc.vector.tensor_copy(vU1[:, :, :, 4:7], vmo[:, :, :, 61:64])
        nc.vector.reduce_sum(
            vU1[:, :, :, 3], vmo[:, :, :, 3:61], axis=mybir.AxisListType.X
        )
        nc.vector.tensor_copy(vUc[:, :, 0:3], vM[:, :, 0:3])
        nc.vector.tensor_copy(vUc[:, :, 4:7], vM[:, :, 61:64])
        nc.vector.reduce_sum(
            vUc[:, :, 3], vM[:, :, 3:61], axis=mybir.AxisListType.X
        )

        # --- d-half 1 encoder (DMA-paced) ---
        enc_chunk(1, 0, 12)
        enc_mms(1, 0)
        ka(4, 128)
        enc_chunk(1, 12, 12)
        enc_mms(1, 12)
        ka(2, 128)
        enc_chunk(1, 24, 12)
        enc_mms(1, 24)
        ka(5, 128)
        enc_chunk(1, 36, 9)
        ka(4, 128)
        enc_mms(1, 36)
        enc_chunk(1, 45, 3)
        # post-stream DMAs: consts, w1 kd0 half, w1 kd1+cross, stats, w2+w3
        nc.sync.dma_start(cons[:], a_in["consts"])
        nc.sync.dma_start(wc[:, 0 : 4 * 256], a_in["wconv"][:, 0 : 4 * 256])
        nc.sync.dma_start(wc[:, 4 * 256 : 13 * 256], a_in["wconv"][:, 4 * 256 : 13 * 256])
        nc.sync.dma_start(
            wc[:, 13 * 256 : NP1 * 256], a_in["wconv"][:, 13 * 256 : NP1 * 256]
        )
        nc.sync.dma_start(big2_sb[:], a_in["big2"])
        nc.sync.dma_start(wc[:, W2BASE:WCCOLS], a_in["wconv"][:, W2BASE:WCCOLS])
        nc.tensor.matmul(
            featsT_ps[1][:], scb[0:1, 256:268], scb[0:1, 128:256],
            start=False, stop=False,
        )
        enc_mms(1, 45, stop=True)
        nc.vector.tensor_copy(featsTb[:, 128:256], featsT_ps[1][:])

        # --- bcast kd1 (critical h0 written by DVE+Act in parallel) ---
        bcs = []
        for h in range(2):
            bc = ps_bc.tile([128, 486], F32, name=f"bc1{h}", tag="bc")
            nc.tensor.matmul(
                bc[:], featsTb[:, 128:256], emat[:, h * 486 : (h + 1) * 486],
                start=True, stop=True,
            )
            bcs.append(bc)
        nc.scalar.activation(
            xpad1[:, 972 : 972 + 243], bcs[0][:, 0:243], AF.Identity
        )
        nc.vector.tensor_copy(xpad1[:, 972 + 243 : 972 + 486], bcs[0][:, 243:486])
        nc.scalar.activation(
            xpad1[:, 972 + 486 : 972 + 729], bcs[1][:, 0:243], AF.Identity
        )
        nc.vector.tensor_copy(xpad1[:, 972 + 729 : 972 + 972], bcs[1][:, 243:486])
        # conv1-kd0 pair-major: fills the PE idle under the bcast/w1 DMAs
        for pi, (ta, tb, pstride) in enumerate(PAIRS1L):
            for si, sp in enumerate(SPLITS):
                t0, nt = sp
                rhs = _win(xpad1[:], tap_off(ta) + t0 * FLAT, pstride, nt)
                nc.tensor.matmul(
                    p1[si][:], w1v_kd0[pi], rhs,
                    start=(pi == 0), stop=False, perf_mode=DR,
                )

        # --- conv chain: cross-stage wavefront ---
        kd1_offs = [0] + [972] * 13
        pairs_kd1 = [((2, 2, 2), (2, 2, 2), 972)] + PAIRS1L
        p2 = [ps_cv.tile([64, (9 * nt - 2) * 7], F32, name=f"p2{si}", tag="cv")
              for si, (t0, nt) in enumerate(SPLITS2)]
        p3 = [ps_cv.tile([3, (9 * nt - 2) * 7], F32, name=f"p3{si}", tag="cv")
              for si, (t0, nt) in enumerate(SPLITS)]

        def w2view(pi):
            return wc[:, W2BASE + pi * 128 : W2BASE + (pi + 1) * 128].rearrange(
                "p (a m) -> p a m", a=2
            )

        def w3view(pi):
            return wc[0:64, W3BASE + pi * 32 : W3BASE + (pi + 1) * 32].rearrange(
                "p (a m) -> p a m", a=2
            )[:, :, 0:3]

        w2vs = [w2view(pi) for pi in range(14)]
        w3vs = [w3view(pi) for pi in range(14)]
        zoffs = [0] * 14

        def c1(si):
            conv_chain(p1[si], SPLITS[si], pairs_kd1, w1v_kd1, xpad1, kd1_offs,
                       None, False, True)

        def a1(si):
            t0, nt = SPLITS[si]
            pin = AP(p1[si].tensor, p1[si].offset,
                     [list(p1[si][:].ap[0]), [63, nt], [7, 7], [1, 7]])
            nc.scalar.activation(
                _xwin(xpad2[:], t0, nt), pin,
                AF.Relu, bias=b1_sb, scale=1.0 / S,
            )

        def c2(si):
            conv_chain(p2[si], SPLITS2[si], PAIRS23, w2vs, xpad2, zoffs, None,
                       True, True)

        def a2(si):
            t0, nt = SPLITS2[si]
            pin = AP(p2[si].tensor, p2[si].offset,
                     [list(p2[si][:].ap[0]), [63, nt], [7, 7], [1, 7]])
            nc.scalar.activation(
                _xwin(xpad3[:], t0, nt, 64), pin,
                AF.Relu, bias=b2_sb, scale=1.0 / S,
            )

        def c3(si):
            conv_chain(p3[si], SPLITS[si], PAIRS23, w3vs, xpad3, zoffs, 64,
                       True, True)

        def a3(si):
            # raw S*conv3 copied out; the host applies /S and +b3 in assembly
            t0, nt = SPLITS[si]
            pin = AP(p3[si].tensor, p3[si].offset,
                     [list(p3[si][:].ap[0]), [63, nt], [7, 7], [1, 7]])
            nc.scalar.activation(
                recon_sb[:, t0 * 49 : (t0 + nt) * 49], pin, AF.Identity
            )

        def pvout():
            nc.tensor.matmul(pv1, rhT_sb, U1[:], start=True, stop=True)
            nc.tensor.matmul(pvc, rhT_sb, Uc[:], start=True, stop=True)
            nc.vector.tensor_copy(outv[0:14, 0:84], pv1)
            nc.vector.tensor_copy(outv[0:14, 84:112], pvc)
            nc.gpsimd.dma_start(a_out["outv"], outv[:])

        c1(0); a1(0)
        c1(1); a1(1)
        c1(2); a1(2)
        c2(0); a2(0)
        c2(1); a2(1)
        pvout()
        c2(2); a2(2)
        c3(0); a3(0)
        c3(1); a3(1)
        c3(2); a3(2)
        nc.sync.dma_start(a_out["recon"], recon_sb[:])


_CACHE = {}


def _build():
    if "nc" in _CACHE:
        return _CACHE["nc"]
    nc = bacc.Bacc("TRN2", target_bir_lowering=False, debug=False)
    a_in = {}

    def din(name, shape, dt):
        a_in[name] = nc.dram_tensor(name, shape, dt, kind="ExternalInput").ap()

    din("sflag", (1, 272), BF16)
    din("consts", (128, 20), F32)
    din("ebmat", (12, 972), BF16)
    din("xt", (128, 1536), FP8)
    din("wenc", (128, 24576), FP8)
    din("wconv", (128, WCCOLS), FP8)
    din("big2", (128, 1040), BF16)
    a_out = {}
    for name, shape in [("recon", (3, NV)), ("outv", (128, 120))]:
        a_out[name] = nc.dram_tensor(name, shape, F32, kind="ExternalOutput").ap()
    _emit(nc, a_in, a_out)
    nc.compile()
    _CACHE["nc"] = nc
    return nc


def make_in_maps(obs_strip, mask, W_enc, b_enc, w1, b1, w2, b2, w3, b3):
    import ml_dtypes

    fp8 = ml_dtypes.float8_e4m3
    bf16 = ml_dtypes.bfloat16

    obs_strip = np.ascontiguousarray(obs_strip, dtype=np.float32)
    mask_f = np.ascontiguousarray(mask).astype(np.float32)

    # ---- shared tensors ----
    sbenc = (S * np.asarray(b_enc, np.float32)).reshape(256)

    rh = np.zeros((NCLS, 64), np.float32)
    for i in range(NCLS):
        rh[i, CLS_BOUNDS[i] : CLS_BOUNDS[i + 1]] = 1.0
    rhT = np.zeros((128, 14), np.float32)
    for u in range(2):
        rhT[u * 64 : (u + 1) * 64, u * 7 : (u + 1) * 7] = rh.T
    consts = np.zeros((128, 20), np.float32)
    consts[:, 0] = np.asarray(b1, np.float32)
    consts[0:64, 1] = np.asarray(b2, np.float32)
    consts[0:3, 2] = np.asarray(b3, np.float32)
    consts[:, 3:17] = rhT

    ebmat = np.zeros((12, 972), np.float32)
    for t in range(12):
        blk = ebmat[t, t * FLAT : (t + 1) * FLAT].reshape(9, 9)
        blk[1:8, 1:8] = 1.0 / S
    ebmat = ebmat.astype(bf16)

    wenc = (
        (S * np.asarray(W_enc, np.float32))
        .astype(fp8)
        .reshape(2, 128, 48, 2, 128)  # [dh, dl, g, gp, p]
        .transpose(0, 2, 4, 3, 1)  # [dh, g, p, gp, dl]
        .reshape(2, 48, 128, 2, 128)  # [dh, g, p, gp, dl]
        .transpose(2, 0, 1, 3, 4)  # [p, dh, g, gp, dl]
        .reshape(128, 24576)
    )
    wenc = np.ascontiguousarray(wenc)

    w1s = (S * np.asarray(w1, np.float32)).astype(fp8).astype(np.float32)
    w2s = (S * np.asarray(w2, np.float32)).astype(fp8).astype(np.float32)
    w3s = (S * np.asarray(w3, np.float32)).astype(fp8).astype(np.float32)
    wconv = np.zeros((128, WCCOLS), np.float32)
    # conv1: kd0 locals, kd1 locals, cross
    pair_list1 = (
        [(a, b, 0, 0) for a, b, _ in PAIRS1L]
        + [(a, b, 1, 1) for a, b, _ in PAIRS1L]
        + [((2, 2, 2), (2, 2, 2), 0, 1)]
    )
    for pi, (ta, tb, kda, kdb) in enumerate(pair_list1):
        blk = wconv[:, pi * 256 : (pi + 1) * 256].reshape(128, 2, 128)
        blk[:, 0, :] = w1s[:, kda * 128 : (kda + 1) * 128, ta[0], ta[1], ta[2]].T
        blk[:, 1, :] = w1s[:, kdb * 128 : (kdb + 1) * 128, tb[0], tb[1], tb[2]].T
    for pi, (ta, tb, _) in enumerate(PAIRS23):
        blk = wconv[:, W2BASE + pi * 128 : W2BASE + (pi + 1) * 128].reshape(
            128, 2, 64
        )
        blk[:, 0, :] = w2s[:, :, ta[0], ta[1], ta[2]].T
        if tb is not None:
            blk[:, 1, :] = w2s[:, :, tb[0], tb[1], tb[2]].T
        blk3 = wconv[0:64, W3BASE + pi * 32 : W3BASE + (pi + 1) * 32].reshape(
            64, 2, 16
        )
        blk3[:, 0, 0:3] = w3s[:, :, ta[0], ta[1], ta[2]].T
        if tb is not None:
            blk3[:, 1, 0:3] = w3s[:, :, tb[0], tb[1], tb[2]].T
    wconv = wconv.astype(fp8)

    shared = {
        "consts": consts,
        "ebmat": ebmat,
        "wenc": wenc,
        "wconv": wconv,
    }

    # ---- per-core tensors ----
    masked = obs_strip * (1.0 - mask_f[:, :, None, :, :])  # [B,T,C,H,W]
    in_maps = []
    for core in range(NCORES):
        b, th = core // 2, core % 2
        base = -1 if th == 0 else 5

        # xt: 12 local frame slots of masked obs, fp8, pixel-major
        win = np.zeros((12, X), np.float32)
        for t in range(12):
            fr = base + t
            if 0 <= fr < T:
                win[t] = masked[b, fr].reshape(X)
        arr = win.astype(fp8).reshape(12, 96, 128).transpose(2, 1, 0)  # [p,ki,t]
        xt = np.zeros((128, 96, 16), fp8)
        xt[:, :, 0:12] = arr
        xt = np.ascontiguousarray(xt.reshape(128, 1536))

        sflag = np.zeros((1, 272), np.float32)
        sflag[0, 0:256] = sbenc
        for t in range(12):
            if 0 <= base + t < T:
                sflag[0, 256 + t] = 1.0
        sflag = sflag.astype(bf16)

        # big2: own-half obs + mask in stat layout, bf16
        tt0 = th * 4
        ob = obs_strip[b].reshape(8, 2, C, H, W)[tt0 : tt0 + 4]  # [4,2,C,H,W]
        mk = mask_f[b].reshape(8, 2, H, W)[tt0 : tt0 + 4]  # [4,2,H,W]
        big2 = np.zeros((128, 1040), np.float32)
        big2[:, 0:768] = ob.transpose(1, 3, 0, 2, 4).reshape(128, 768)
        big2[:, 768:1024] = mk.transpose(1, 2, 0, 3).reshape(128, 256)
        big2 = big2.astype(bf16)

        in_maps.append({"xt": xt, "sflag": sflag, "big2": big2, **shared})
    return in_maps


def assemble(results, b3_glob):
    total_sq = 0.0
    total_cnt = 0.0
    for core in range(NCORES):
        b, th = core // 2, core % 2
        r = results[core]
        recon = r["recon"].astype(np.float64).reshape(3, NT, 7, 7) / S
        recon += np.asarray(b3_glob, np.float64)[:, None, None, None]
        sl = slice(0, 8) if th == 0 else slice(2, 10)
        rt = recon[:, sl].transpose(1, 0, 2, 3)  # [8 frames, c, i, j]
        outv = r["outv"].astype(np.float64)
        pv1 = outv[0:14, 0:84].reshape(2, 7, TTL, C, NCLS)  # [u, i, tt, c, j]
        pvc = outv[0:14, 84:112].reshape(2, 7, TTL, NCLS)  # [u, i, tt, j]
        s2 = float(outv[:, 112].sum())
        # local frame (of 8) = tt*2 + u
        s1 = np.zeros((8, C, NCLS, NCLS))
        cnt = np.zeros((8, NCLS, NCLS))
        for u in range(2):
            s1[u::2] = pv1[u].transpose(1, 2, 0, 3)  # [tt, c, i, j]
            cnt[u::2] = pvc[u].transpose(1, 0, 2)  # [tt, i, j]
        total_sq += float(
            (rt * rt * cnt[:, None]).sum() - 2.0 * (rt * s1).sum() + s2
        )
        total_cnt += float(cnt.sum())
    loss = total_sq / max(total_cnt * C, 1.0)
    return np.float32(loss)


def kernel(**inputs):
    nc = _build()
    in_maps = make_in_maps(**inputs)
    res = bass_utils.run_bass_kernel_spmd(nc, in_maps, core_ids=list(range(NCORES)))
    _CACHE["last_res"] = res
    return assemble(res.results, np.asarray(inputs["b3"], np.float64))


if __name__ == "__main__":
    pass


# revision 50
# speedup vs baseline: 1.0086x; 1.0086x over previous
"""Masked video loss kernel for TRN2 (8 NeuronCores, SPMD) — fp8 DoubleRow.

Exploits: (1) the decoder input is spatially constant, so the three SAME
3x3x3 convs take at most 7x7 distinct values per (b,c,t) — evaluated
exactly on a 7x7 class grid; (2) fp8 DoubleRow matmuls (two 128-row
contraction planes per pass at 0.5 cycles/output-col); (3) a t-split
across the two cores of each batch: core th computes recon only for its
8 frames (plus conv halo), with the shift encoded purely in host-side
data placement so the SPMD program is identical on all cores.

Per-core program:
  encoder  feats[t,d] = (masked obs)·W_enc^T + b_enc for 12 local frame
           slots (11 real + 1 zero pad), fp8 DR over 96 k-chunks, with
           d-halves pipelined against the W_enc DMA stream.
  bcast    xpad1[d,(t,9,9)] = feats broadcast to the padded class grid,
           done on the PE with a constant 0/1-ish matrix E (entries 1/S,
           which also undoes the fp8 weight scaling S).
  conv1-3  each SAME conv = 27 taps paired into fp8-DR plane pairs over
           overlapping windows of the padded (13-slot x 9 x 9) volume.
           Weight scale S folded out via activation scale=1/S.
  stats    masked-MSE per-class statistics (cnt, s1, s2) from bf16 obs,
           own t-half only, on DVE + 2 small PE matmuls.
Host assembles the scalar loss from per-class tensors (~10k flops).

mask zeroing: sum_masked (r-o)^2 = r^2*cnt - 2 r*s1 + s2 per class.
"""

import sys

sys.path.insert(0, "/opt/trn_rl_repo")

from contextlib import ExitStack  # noqa: E402

import numpy as np  # noqa: E402

import concourse.bacc as bacc  # noqa: E402
import concourse.mybir as mybir  # noqa: E402
import concourse.tile as tile  # noqa: E402
from concourse import bass_utils  # noqa: E402
from concourse.ap import AP  # noqa: E402

B, T, C, H, W = 4, 16, 3, 64, 64
D = 256
X = C * H * W  # 12288
NCORES = 8
S = 64.0  # fp8 weight scale (keeps 0.02-scale weights out of e4m3 subnormals)

F32 = mybir.dt.float32
BF16 = mybir.dt.bfloat16
FP8 = mybir.dt.float8e4
DR = mybir.MatmulPerfMode.DoubleRow
AF = mybir.ActivationFunctionType

NF = 12  # local feats slots (11 real frames + 1 zero pad)
NT = 10  # conv out t-slots per stage
S23 = 13  # xpad2/xpad3 t-slots (extra zero slot for the Z-plane trick)
FLAT = 81  # 9x9 padded spatial grid, flattened
NV = NT * 49  # 490 valid out cols per stage

# conv out t-splits (t0, nt): each split's rhs free = 2*nt*61 <= 512.
# conv2's tiny first split lets it start after conv1's first act alone.
SPLITS = [(0, 4), (4, 4), (8, 2)]
SPLITS2 = [(0, 4), (4, 4), (8, 2)]

# DR plane-pair tables. Pair = (tapA, tapB, plane_stride); tapB None => zero
# weights in plane B (rhs reads base+stride window, in-bounds by design).
KHW = [(kh, kw) for kh in range(3) for kw in range(3)]


def tap_off(t):
    kt, kh, kw = t
    return kt * FLAT + kh * 9 + kw


# conv2/conv3: 27 taps -> 14 pairs (one Z plane)
PAIRS23 = (
    [((0, kh, kw), (1, kh, kw), FLAT) for kh, kw in KHW]
    + [((2, 0, kw), (2, 1, kw), 9) for kw in range(3)]
    + [((2, 2, 0), (2, 2, 1), 1), ((2, 2, 2), None, FLAT)]
)
# conv1 per-kd local pairs: 13 (covers all taps except (2,2,2))
PAIRS1L = (
    [((0, kh, kw), (1, kh, kw), FLAT) for kh, kw in KHW]
    + [((2, 0, kw), (2, 1, kw), 9) for kw in range(3)]
    + [((2, 2, 0), (2, 2, 1), 1)]
)
NP1 = 2 * len(PAIRS1L) + 1  # 27: kd0 locals + kd1 locals + one cross-kd pair

CLS_BOUNDS = [0, 1, 2, 3, H - 3, H - 2, H - 1, H]
NCLS = 7
TTL = 4  # local tt pairs (own t-half = 8 frames = 4 frame-pairs)

W1COLS = NP1 * 256  # 6912
W2BASE = W1COLS
W2COLS = len(PAIRS23) * 128  # 1792
W3BASE = W2BASE + W2COLS
W3COLS = len(PAIRS23) * 32  # padded for DR stride%16
WCCOLS = W3BASE + W3COLS


def _win(tile_ap, off, pstride, nt, npart=None):
    """rhs window AP [p][plane 2][t nt][flat 61]: 3 free dims (HW ISA limit).

    The 61-span covers the 7x7 window in the 9-col row layout, including 12
    junk columns (j in {7,8} positions) that downstream activations skip."""
    pdim = [tile_ap.ap[0][0], npart or tile_ap.ap[0][1]]
    return AP(
        tile_ap.tensor,
        tile_ap.offset + off,
        [pdim, [pstride, 2], [9, 9 * nt - 2], [1, 7]],
    )


def _xwin(tile_ap, t0, nt, npart=None):
    """xpad write AP [p][t nt][i 7][j 7] at interior of slots t0+1.."""
    pdim = [tile_ap.ap[0][0], npart or tile_ap.ap[0][1]]
    return AP(
        tile_ap.tensor,
        tile_ap.offset + (t0 + 1) * FLAT + 10,
        [pdim, [FLAT, nt], [9, 7], [1, 7]],
    )


def _emit(nc, a_in, a_out):
    ctx = ExitStack()
    tc = tile.TileContext(nc)
    with tc, ctx:
        io = ctx.enter_context(tc.tile_pool(name="io", bufs=1))
        wpool = ctx.enter_context(tc.tile_pool(name="wpool", bufs=1))
        ps_enc = ctx.enter_context(tc.tile_pool(name="ps_enc", bufs=2, space="PSUM"))
        ps_bc = ctx.enter_context(tc.tile_pool(name="ps_bc", bufs=2, space="PSUM"))
        ps_cv = ctx.enter_context(tc.tile_pool(name="ps_cv", bufs=3, space="PSUM"))
        ps_wm = ctx.enter_context(tc.tile_pool(name="ps_wm", bufs=1, space="PSUM"))

        # ---------------- input DMAs ----------------
        # warm-up operand: Pool memset, no DMA dependency
        wz = io.tile([1, 256], BF16)
        nc.gpsimd.memset(wz[:], 0.0)
        xt = io.tile([128, 2048], FP8)
        nc.sync.dma_start(xt[:], a_in["xt"])
        # t padded 12->16: DoubleRow ldweights needs plane stride % 16 == 0
        xt_v = xt[:, 0:1536].rearrange("p (g a t) -> p g a t", g=48, a=2)[:, :, :, 0:12]
        # first two wenc-h1 k-pairs ride inside xt: the stream start is gated
        # by the Act engine's c0 request, so these bytes are latency-free and
        # shorten the wenc stream tail by the same amount
        xtw = xt[:, 1536:2048].rearrange("p (g a d) -> p g a d", g=2, a=2)
        scb = io.tile([1, 272], BF16)  # [0:256) S*b_enc | [256:268) flag
        nc.sync.dma_start(scb[:], a_in["sflag"])

        # persistent SBUF tiles
        wc = io.tile([128, WCCOLS], FP8)
        featsTb = io.tile([12, 256], BF16)
        xpad1 = io.tile([128, 2 * NF * FLAT], FP8)  # [kd 2][t 12][81]
        xpad2 = io.tile([128, S23 * FLAT], FP8)
        xpad3 = io.tile([64, S23 * FLAT], FP8)
        recon_sb = io.tile([3, NV], F32)
        outv = io.tile([128, 120], F32)
        big2_sb = io.tile([128, 1040], BF16)
        cons = io.tile([128, 20], F32)
        emat = io.tile([12, 972], BF16)
        b1_sb = cons[:, 0:1]
        b2_sb = cons[0:64, 1:2]
        b3_sb = cons[0:3, 2:3]
        rhT_sb = cons[:, 3:17]
        nc.gpsimd.memset(xpad2[:], 0.0)
        nc.gpsimd.memset(xpad3[:], 0.0)
        nc.gpsimd.memset(outv[:], 0.0)

        # warm/keep-alive PSUM bank also hosts the tiny stats matmuls
        warm = ps_wm.tile([14, 368], F32)
        pv1 = warm[0:14, 256:340]
        pvc = warm[0:14, 340:368]

        def ka(n, cols=256):
            for _ in range(n):
                nc.tensor.matmul(
                    warm[0:2, 0:cols], wz[0:1, 0:2], wz[0:1, 0:cols],
                    start=True, stop=True,
                )

        # ---------------- PE warm-up during first DMAs ----------------
        ka(12)

        # ---------------- encoder, d-half pipelined against W_enc DMA ----
        featsT_ps = [None, None]
        wtiles = {}

        def enc_chunk(dh, g0, ng, eng=None):
            wk = wpool.tile(
                [128, ng * 256], FP8, name=f"wk{dh}{g0}", tag=f"wk{dh}{g0}"
            )
            base = dh * 12288 + g0 * 256
            (eng or nc.sync).dma_start(
                wk[:], a_in["wenc"][:, base : base + ng * 256]
            )
            wtiles[(dh, g0)] = (wk, ng)

        def enc_mms(dh, g0, stop=False):
            wk, ng = wtiles[(dh, g0)]
            wv = wk[:].rearrange("p (g a d) -> p g a d", g=ng, a=2)
            for gl in range(ng):
                g = g0 + gl
                nc.tensor.matmul(
                    featsT_ps[dh][:],
                    xt_v[:, g, :, :],
                    wv[:, gl, :, :],
                    start=(g == 0),
                    stop=(stop and gl == ng - 1),
                    perf_mode=DR,
                )

        def conv_chain(ps_t, split, pairs, wviews, xtile, kd_offs, npart,
                       start, stop):
            t0, nt = split
            n = len(pairs)
            for pi, (ta, tb, pstride) in enumerate(pairs):
                rhs = _win(
                    xtile[:], kd_offs[pi] + tap_off(ta) + t0 * FLAT, pstride,
                    nt, npart,
                )
                nc.tensor.matmul(
                    ps_t[:], wviews[pi], rhs,
                    start=(start and pi == 0),
                    stop=(stop and pi == n - 1),
                    perf_mode=DR,
                )

        # --- d-half 0 encoder (DMA-paced) ---
        featsT_ps[0] = ps_enc.tile([12, 128], F32, name="fe0", tag="fe")
        enc_chunk(0, 0, 12, eng=nc.scalar)
        enc_chunk(0, 12, 12)
        enc_mms(0, 0)
        ka(3, 128)
        enc_chunk(0, 24, 12)
        enc_mms(0, 12)
        ka(3, 128)
        enc_chunk(0, 36, 12)
        enc_mms(0, 24)
        ka(3, 128)
        nc.sync.dma_start(emat[:], a_in["ebmat"])
        # b_enc via per-core real-frame flag row (zero on pad frames)
        nc.tensor.matmul(
            featsT_ps[0][:], scb[0:1, 256:268], scb[0:1, 0:128],
            start=False, stop=False,
        )
        enc_mms(0, 36, stop=True)
        nc.vector.tensor_copy(featsTb[:, 0:128], featsT_ps[0][:])

        # --- bcast kd0 ---
        for h in range(2):
            bc = ps_bc.tile([128, 486], F32, name=f"bc0{h}", tag="bc")
            nc.tensor.matmul(
                bc[:], featsTb[:, 0:128], emat[:, h * 486 : (h + 1) * 486],
                start=True, stop=True,
            )
            nc.scalar.activation(
                xpad1[:, h * 486 : (h + 1) * 486], bc[:], AF.Identity
            )
        ka(3)
        featsT_ps[1] = ps_enc.tile([12, 128], F32, name="fe1", tag="fe")

        # --- conv1 kd0 setup (mms run post-stream, in the bcast-hop idle) ---
        p1 = [ps_cv.tile([128, (9 * nt - 2) * 7], F32, name=f"p1{si}", tag="cv")
              for si, (t0, nt) in enumerate(SPLITS)]

        def w1view(pi):
            return wc[:, pi * 256 : (pi + 1) * 256].rearrange(
                "p (a m) -> p a m", a=2
            )

        w1v_kd0 = [w1view(pi) for pi in range(13)]
        w1v_kd1 = [w1view(26)] + [w1view(13 + pi) for pi in range(13)]

        # --- stats payload + stats compute (runs mid-stream on DVE) ---
        obs_st = big2_sb[:, 0:768]
        mask_st = big2_sb[:, 768:1024]
        vO = obs_st.rearrange("p (tt c w) -> p tt c w", tt=TTL, c=C)
        vM = mask_st.rearrange("p (tt w) -> p tt w", tt=TTL)
        mo = io.tile([128, TTL * C * W], BF16)
        vmo = mo[:].rearrange("p (tt c w) -> p tt c w", tt=TTL, c=C)
        for c in range(C):
            nc.vector.tensor_mul(vmo[:, :, c, :], vO[:, :, c, :], vM[:])
        mo2 = io.tile([128, TTL * C * W], BF16)
        nc.vector.tensor_mul(mo2[:], mo[:], obs_st)
        nc.vector.reduce_sum(outv[:, 112:113], mo2[:], axis=mybir.AxisListType.X)
        U1 = io.tile([128, TTL * C * NCLS], F32)
        vU1 = U1[:].rearrange("p (tt c j) -> p tt c j", tt=TTL, c=C)
        Uc = io.tile([128, TTL * NCLS], F32)
        vUc = Uc[:].rearrange("p (tt j) -> p tt j", tt=TTL)
        nc.vector.tensor_copy(vU1[:, :, :, 0:3], vmo[:, :, :, 0:3])
        nc.vector.tensor_copy(vU1[:, :, :, 4:7], vmo[:, :, :, 61:64])
        nc.vector.reduce_sum(
            vU1[:, :, :, 3], vmo[:, :, :, 3:61], axis=mybir.AxisListType.X
        )
        nc.vector.tensor_copy(vUc[:, :, 0:3], vM[:, :, 0:3])
        nc.vector.tensor_copy(vUc[:, :, 4:7], vM[:, :, 61:64])
        nc.vector.reduce_sum(
            vUc[:, :, 3], vM[:, :, 3:61], axis=mybir.AxisListType.X
        )

        # --- d-half 1 encoder (DMA-paced; first 2 k-pairs come from xt) ---
        for gl in range(2):
            nc.tensor.matmul(
                featsT_ps[1][:],
                xt_v[:, gl, :, :],
                xtw[:, gl, :, :],
                start=(gl == 0),
                stop=False,
                perf_mode=DR,
            )
        enc_chunk(1, 2, 12)
        enc_mms(1, 2)
        ka(4, 128)
        enc_chunk(1, 14, 12)
        enc_mms(1, 14)
        ka(2, 128)
        enc_chunk(1, 26, 12)
        enc_mms(1, 26)
        ka(5, 128)
        enc_chunk(1, 38, 7)
        ka(4, 128)
        enc_mms(1, 38)
        enc_chunk(1, 45, 3)
        # post-stream DMAs: consts, w1 kd0 half, w1 kd1+cross, stats, w2+w3
        nc.sync.dma_start(cons[:], a_in["consts"])
        nc.sync.dma_start(wc[:, 0 : 4 * 256], a_in["wconv"][:, 0 : 4 * 256])
        nc.sync.dma_start(wc[:, 4 * 256 : 13 * 256], a_in["wconv"][:, 4 * 256 : 13 * 256])
        nc.sync.dma_start(
            wc[:, 13 * 256 : NP1 * 256], a_in["wconv"][:, 13 * 256 : NP1 * 256]
        )
        nc.sync.dma_start(big2_sb[:], a_in["big2"])
        nc.sync.dma_start(wc[:, W2BASE:WCCOLS], a_in["wconv"][:, W2BASE:WCCOLS])
        nc.tensor.matmul(
            featsT_ps[1][:], scb[0:1, 256:268], scb[0:1, 128:256],
            start=False, stop=False,
        )
        enc_mms(1, 45, stop=True)
        nc.vector.tensor_copy(featsTb[:, 128:256], featsT_ps[1][:])

        # --- bcast kd1 (critical h0 written by DVE+Act in parallel) ---
        bcs = []
        for h in range(2):
            bc = ps_bc.tile([128, 486], F32, name=f"bc1{h}", tag="bc")
            nc.tensor.matmul(
                bc[:], featsTb[:, 128:256], emat[:, h * 486 : (h + 1) * 486],
                start=True, stop=True,
            )
            bcs.append(bc)
        nc.scalar.activation(
            xpad1[:, 972 : 972 + 243], bcs[0][:, 0:243], AF.Identity
        )
        nc.vector.tensor_copy(xpad1[:, 972 + 243 : 972 + 486], bcs[0][:, 243:486])
        nc.scalar.activation(
            xpad1[:, 972 + 486 : 972 + 729], bcs[1][:, 0:243], AF.Identity
        )
        nc.vector.tensor_copy(xpad1[:, 972 + 729 : 972 + 972], bcs[1][:, 243:486])
        # conv1-kd0 pair-major: fills the PE idle under the bcast/w1 DMAs
        for pi, (ta, tb, pstride) in enumerate(PAIRS1L):
            for si, sp in enumerate(SPLITS):
                t0, nt = sp
                rhs = _win(xpad1[:], tap_off(ta) + t0 * FLAT, pstride, nt)
                nc.tensor.matmul(
                    p1[si][:], w1v_kd0[pi], rhs,
                    start=(pi == 0), stop=False, perf_mode=DR,
                )

        # --- conv chain: cross-stage wavefront ---
        kd1_offs = [0] + [972] * 13
        pairs_kd1 = [((2, 2, 2), (2, 2, 2), 972)] + PAIRS1L
        p2 = [ps_cv.tile([64, (9 * nt - 2) * 7], F32, name=f"p2{si}", tag="cv")
              for si, (t0, nt) in enumerate(SPLITS2)]
        p3 = [ps_cv.tile([3, (9 * nt - 2) * 7], F32, name=f"p3{si}", tag="cv")
              for si, (t0, nt) in enumerate(SPLITS)]

        def w2view(pi):
            return wc[:, W2BASE + pi * 128 : W2BASE + (pi + 1) * 128].rearrange(
                "p (a m) -> p a m", a=2
            )

        def w3view(pi):
            return wc[0:64, W3BASE + pi * 32 : W3BASE + (pi + 1) * 32].rearrange(
                "p (a m) -> p a m", a=2
            )[:, :, 0:3]

        w2vs = [w2view(pi) for pi in range(14)]
        w3vs = [w3view(pi) for pi in range(14)]
        zoffs = [0] * 14

        def c1(si):
            conv_chain(p1[si], SPLITS[si], pairs_kd1, w1v_kd1, xpad1, kd1_offs,
                       None, False, True)

        def a1(si):
            t0, nt = SPLITS[si]
            pin = AP(p1[si].tensor, p1[si].offset,
                     [list(p1[si][:].ap[0]), [63, nt], [7, 7], [1, 7]])
            nc.scalar.activation(
                _xwin(xpad2[:], t0, nt), pin,
                AF.Relu, bias=b1_sb, scale=1.0 / S,
            )

        def c2(si):
            conv_chain(p2[si], SPLITS2[si], PAIRS23, w2vs, xpad2, zoffs, None,
                       True, True)

        def a2(si):
            t0, nt = SPLITS2[si]
            pin = AP(p2[si].tensor, p2[si].offset,
                     [list(p2[si][:].ap[0]), [63, nt], [7, 7], [1, 7]])
            nc.scalar.activation(
                _xwin(xpad3[:], t0, nt, 64), pin,
                AF.Relu, bias=b2_sb, scale=1.0 / S,
            )

        def c3(si):
            conv_chain(p3[si], SPLITS[si], PAIRS23, w3vs, xpad3, zoffs, 64,
                       True, True)

        def a3(si):
            # raw S*conv3 copied out; the host applies /S and +b3 in assembly
            t0, nt = SPLITS[si]
            pin = AP(p3[si].tensor, p3[si].offset,
                     [list(p3[si][:].ap[0]), [63, nt], [7, 7], [1, 7]])
            nc.scalar.activation(
                recon_sb[:, t0 * 49 : (t0 + nt) * 49], pin, AF.Identity
            )

        def pvout():
            nc.tensor.matmul(pv1, rhT_sb, U1[:], start=True, stop=True)
            nc.tensor.matmul(pvc, rhT_sb, Uc[:], start=True, stop=True)
            nc.vector.tensor_copy(outv[0:14, 0:84], pv1)
            nc.vector.tensor_copy(outv[0:14, 84:112], pvc)
            nc.gpsimd.dma_start(a_out["outv"], outv[:])

        c1(0); a1(0)
        c1(1); a1(1)
        c1(2); a1(2)
        c2(0); a2(0)
        c2(1); a2(1)
        pvout()
        c2(2); a2(2)
        c3(0); a3(0)
        c3(1); a3(1)
        c3(2); a3(2)
        nc.sync.dma_start(a_out["recon"], recon_sb[:])


_CACHE = {}


def _build():
    if "nc" in _CACHE:
        return _CACHE["nc"]
    nc = bacc.Bacc("TRN2", target_bir_lowering=False, debug=False)
    a_in = {}

    def din(name, shape, dt):
        a_in[name] = nc.dram_tensor(name, shape, dt, kind="ExternalInput").ap()

    din("sflag", (1, 272), BF16)
    din("consts", (128, 20), F32)
    din("ebmat", (12, 972), BF16)
    din("xt", (128, 2048), FP8)
    din("wenc", (128, 24576), FP8)
    din("wconv", (128, WCCOLS), FP8)
    din("big2", (128, 1040), BF16)
    a_out = {}
    for name, shape in [("recon", (3, NV)), ("outv", (128, 120))]:
        a_out[name] = nc.dram_tensor(name, shape, F32, kind="ExternalOutput").ap()
    _emit(nc, a_in, a_out)
    nc.compile()
    _CACHE["nc"] = nc
    return nc


def make_in_maps(obs_strip, mask, W_enc, b_enc, w1, b1, w2, b2, w3, b3):
    import ml_dtypes

    fp8 = ml_dtypes.float8_e4m3
    bf16 = ml_dtypes.bfloat16

    obs_strip = np.ascontiguousarray(obs_strip, dtype=np.float32)
    mask_f = np.ascontiguousarray(mask).astype(np.float32)

    # ---- shared tensors ----
    sbenc = (S * np.asarray(b_enc, np.float32)).reshape(256)

    rh = np.zeros((NCLS, 64), np.float32)
    for i in range(NCLS):
        rh[i, CLS_BOUNDS[i] : CLS_BOUNDS[i + 1]] = 1.0
    rhT = np.zeros((128, 14), np.float32)
    for u in range(2):
        rhT[u * 64 : (u + 1) * 64, u * 7 : (u + 1) * 7] = rh.T
    consts = np.zeros((128, 20), np.float32)
    consts[:, 0] = np.asarray(b1, np.float32)
    consts[0:64, 1] = np.asarray(b2, np.float32)
    consts[0:3, 2] = np.asarray(b3, np.float32)
    consts[:, 3:17] = rhT

    ebmat = np.zeros((12, 972), np.float32)
    for t in range(12):
        blk = ebmat[t, t * FLAT : (t + 1) * FLAT].reshape(9, 9)
        blk[1:8, 1:8] = 1.0 / S
    ebmat = ebmat.astype(bf16)

    wenc = (
        (S * np.asarray(W_enc, np.float32))
        .astype(fp8)
        .reshape(2, 128, 48, 2, 128)  # [dh, dl, g, gp, p]
        .transpose(0, 2, 4, 3, 1)  # [dh, g, p, gp, dl]
        .reshape(2, 48, 128, 2, 128)  # [dh, g, p, gp, dl]
        .transpose(2, 0, 1, 3, 4)  # [p, dh, g, gp, dl]
        .reshape(128, 24576)
    )
    wenc = np.ascontiguousarray(wenc)

    w1s = (S * np.asarray(w1, np.float32)).astype(fp8).astype(np.float32)
    w2s = (S * np.asarray(w2, np.float32)).astype(fp8).astype(np.float32)
    w3s = (S * np.asarray(w3, np.float32)).astype(fp8).astype(np.float32)
    wconv = np.zeros((128, WCCOLS), np.float32)
    # conv1: kd0 locals, kd1 locals, cross
    pair_list1 = (
        [(a, b, 0, 0) for a, b, _ in PAIRS1L]
        + [(a, b, 1, 1) for a, b, _ in PAIRS1L]
        + [((2, 2, 2), (2, 2, 2), 0, 1)]
    )
    for pi, (ta, tb, kda, kdb) in enumerate(pair_list1):
        blk = wconv[:, pi * 256 : (pi + 1) * 256].reshape(128, 2, 128)
        blk[:, 0, :] = w1s[:, kda * 128 : (kda + 1) * 128, ta[0], ta[1], ta[2]].T
        blk[:, 1, :] = w1s[:, kdb * 128 : (kdb + 1) * 128, tb[0], tb[1], tb[2]].T
    for pi, (ta, tb, _) in enumerate(PAIRS23):
        blk = wconv[:, W2BASE + pi * 128 : W2BASE + (pi + 1) * 128].reshape(
            128, 2, 64
        )
        blk[:, 0, :] = w2s[:, :, ta[0], ta[1], ta[2]].T
        if tb is not None:
            blk[:, 1, :] = w2s[:, :, tb[0], tb[1], tb[2]].T
        blk3 = wconv[0:64, W3BASE + pi * 32 : W3BASE + (pi + 1) * 32].reshape(
            64, 2, 16
        )
        blk3[:, 0, 0:3] = w3s[:, :, ta[0], ta[1], ta[2]].T
        if tb is not None:
            blk3[:, 1, 0:3] = w3s[:, :, tb[0], tb[1], tb[2]].T
    wconv = wconv.astype(fp8)

    shared = {
        "consts": consts,
        "ebmat": ebmat,
        "wenc": wenc,
        "wconv": wconv,
    }

    # ---- per-core tensors ----
    masked = obs_strip * (1.0 - mask_f[:, :, None, :, :])  # [B,T,C,H,W]
    in_maps = []
    for core in range(NCORES):
        b, th = core // 2, core % 2
        base = -1 if th == 0 else 5

        # xt: 12 local frame slots of masked obs, fp8, pixel-major
        win = np.zeros((12, X), np.float32)
        for t in range(12):
            fr = base + t
            if 0 <= fr < T:
                win[t] = masked[b, fr].reshape(X)
        arr = win.astype(fp8).reshape(12, 96, 128).transpose(2, 1, 0)  # [p,ki,t]
        xt = np.zeros((128, 2048), fp8)
        xt[:, 0:1536] = np.zeros((128, 96, 16), fp8).reshape(128, 1536)
        xtv = xt[:, 0:1536].reshape(128, 96, 16)
        xtv[:, :, 0:12] = arr
        xt[:, 1536:2048] = wenc[:, 12288 : 12288 + 512]
        xt = np.ascontiguousarray(xt)

        sflag = np.zeros((1, 272), np.float32)
        sflag[0, 0:256] = sbenc
        for t in range(12):
            if 0 <= base + t < T:
                sflag[0, 256 + t] = 1.0
        sflag = sflag.astype(bf16)

        # big2: own-half obs + mask in stat layout, bf16
        tt0 = th * 4
        ob = obs_strip[b].reshape(8, 2, C, H, W)[tt0 : tt0 + 4]  # [4,2,C,H,W]
        mk = mask_f[b].reshape(8, 2, H, W)[tt0 : tt0 + 4]  # [4,2,H,W]
        big2 = np.zeros((128, 1040), np.float32)
        big2[:, 0:768] = ob.transpose(1, 3, 0, 2, 4).reshape(128, 768)
        big2[:, 768:1024] = mk.transpose(1, 2, 0, 3).reshape(128, 256)
        big2 = big2.astype(bf16)

        in_maps.append({"xt": xt, "sflag": sflag, "big2": big2, **shared})
    return in_maps


def assemble(results, b3_glob):
    total_sq = 0.0
    total_cnt = 0.0
    for core in range(NCORES):
        b, th = core // 2, core % 2
        r = results[core]
        recon = r["recon"].astype(np.float64).reshape(3, NT, 7, 7) / S
        recon += np.asarray(b3_glob, np.float64)[:, None, None, None]
        sl = slice(0, 8) if th == 0 else slice(2, 10)
        rt = recon[:, sl].transpose(1, 0, 2, 3)  # [8 frames, c, i, j]
        outv = r["outv"].astype(np.float64)
        pv1 = outv[0:14, 0:84].reshape(2, 7, TTL, C, NCLS)  # [u, i, tt, c, j]
        pvc = outv[0:14, 84:112].reshape(2, 7, TTL, NCLS)  # [u, i, tt, j]
        s2 = float(outv[:, 112].sum())
        # local frame (of 8) = tt*2 + u
        s1 = np.zeros((8, C, NCLS, NCLS))
        cnt = np.zeros((8, NCLS, NCLS))
        for u in range(2):
            s1[u::2] = pv1[u].transpose(1, 2, 0, 3)  # [tt, c, i, j]
            cnt[u::2] = pvc[u].transpose(1, 0, 2)  # [tt, i, j]
        total_sq += float(
            (rt * rt * cnt[:, None]).sum() - 2.0 * (rt * s1).sum() + s2
        )
        total_cnt += float(cnt.sum())
    loss = total_sq / max(total_cnt * C, 1.0)
    return np.float32(loss)


def kernel(**inputs):
    nc = _build()
    in_maps = make_in_maps(**inputs)
    res = bass_utils.run_bass_kernel_spmd(nc, in_maps, core_ids=list(range(NCORES)))
    _CACHE["last_res"] = res
    return assemble(res.results, np.asarray(inputs["b3"], np.float64))


if __name__ == "__main__":
    pass
